# revision 16
# baseline (speedup 1.0000x reference)
"""MemoryMoCo (8-view combos + FIFO memory update) on 8 Trainium2 cores.

Sharding: over the queue dim Q. Core k holds rows [k*4096, (k+1)*4096) of
every memory bank. Per core:
  - all 8 combos (i,j): exp(scale * X_i @ Mem_j_slice^T) -> [8, 512, 4096]
  - positive-logit column exp(scale * <X_i, X_j>) (replicated; host takes core 0)
  - FIFO shift: streams its natural-layout mem tiles back out (rows shift by B
    on the host side of the gather), Y tail passes through the device.
Matmuls run in float32r (full-rate fp32 PE mode); norms/logit scaling in fp32.
"""
import numpy as np

import concourse.bass as bass
import concourse.tile as tile
import concourse.mybir as mybir
from concourse import bacc
from concourse import bass_utils
from concourse.masks import make_identity

dt = mybir.dt

N_CORES = 8
B, Q, D = 512, 32768, 256
T = 0.07
COMBOS = ((0, 1), (0, 2), (0, 3), (1, 0), (1, 2), (2, 0), (2, 1), (3, 0))
NC_ = len(COMBOS)
NV = 4                      # views
QC = Q // N_CORES           # 4096 queue rows per core
QB = 512                    # queue rows per main-loop iteration
NQB = QC // QB              # 8 iterations per view
NBT = B // 128              # 4 batch tiles
YT = B // N_CORES           # 64 Y-tail rows per core

_CACHE: dict = {}


def _build(stage=9):
    nc = bacc.Bacc("TRN2", target_bir_lowering=False, debug=False,
                   num_devices=N_CORES)

    x_in = [nc.dram_tensor(f"x{v}", [B, D], dt.float32, kind="ExternalInput")
            for v in range(NV)]
    mem_in = [nc.dram_tensor(f"mem{v}", [QC, D], dt.float32, kind="ExternalInput")
              for v in range(NV)]
    ytail_in = nc.dram_tensor("ytail", [NV, YT, D], dt.float32, kind="ExternalInput")

    outs_chunk = nc.dram_tensor("outs_chunk", [NC_, B, QC], dt.float32,
                                kind="ExternalOutput")
    selfexp = nc.dram_tensor("selfexp", [NBT, 128, NC_], dt.float32,
                             kind="ExternalOutput")
    newmem_chunk = nc.dram_tensor("newmem_chunk", [NV, QC, D], dt.float32,
                                  kind="ExternalOutput")
    ytail_out = nc.dram_tensor("ytail_out", [NV, YT, D], dt.float32,
                               kind="ExternalOutput")

    with tile.TileContext(nc) as tc:
        with (
            tc.tile_pool(name="const", bufs=1) as const_pool,
            tc.tile_pool(name="resident", bufs=1) as res_pool,
            tc.tile_pool(name="scratch", bufs=2) as scratch_pool,
            tc.tile_pool(name="mnat", bufs=6) as mnat_pool,
            tc.tile_pool(name="rhs", bufs=3) as rhs_pool,
            tc.tile_pool(name="osb", bufs=8) as osb_pool,
            tc.tile_pool(name="pst", bufs=2, space="PSUM") as pst_pool,
            tc.tile_pool(name="psmm", bufs=3, space="PSUM") as psmm_pool,
        ):
            ident = const_pool.tile([128, 128], dt.float32)
            make_identity(nc, ident)

            # ---- Y tail passthrough (tiny) ----
            yt_sb = res_pool.tile([YT, NV, D], dt.float32, tag="ytail")
            nc.sync.dma_start(yt_sb, ytail_in.ap().rearrange("v p d -> p v d"))
            nc.sync.dma_start(ytail_out.ap().rearrange("v p d -> p v d"), yt_sb)

            # ---- load X natural: [128, bt, d] with row = bt*128 + p ----
            x_sb = []
            for v in range(NV):
                t = res_pool.tile([128, NBT, D], dt.float32, tag=f"xnat{v}")
                nc.sync.dma_start(t, x_in[v].ap().rearrange("(bt p) d -> p bt d", p=128))
                x_sb.append(t)

            # ---- transpose X on PE -> xT_r [128 d_half, k, b] in f32r ----
            xT_r = []
            if stage >= 2:
                for v in range(NV):
                    t = res_pool.tile([128, 2, B], dt.float32r, tag=f"xtr{v}")
                    xT_r.append(t)
                    for bt in range(NBT):
                        for k in range(2):
                            pt = pst_pool.tile([128, 128], dt.float32, tag="tp")
                            nc.tensor.transpose(
                                pt, x_sb[v][:, bt, k * 128:(k + 1) * 128], ident)
                            nc.vector.tensor_copy(
                                t[:, k, bt * 128:(bt + 1) * 128], pt)

            # ---- norms^2 per view (ACT Square + row-accumulate) ----
            sc_all = res_pool.tile([128, NC_ * NBT], dt.float32, tag="scall")
            if stage >= 3:
                norm2 = []
                for v in range(NV):
                    t = res_pool.tile([128, NBT], dt.float32, tag=f"n2{v}")
                    norm2.append(t)
                    for bt in range(NBT):
                        sc = scratch_pool.tile([128, D], dt.float32, tag="ttr")
                        nc.scalar.activation(
                            sc, x_sb[v][:, bt, :],
                            mybir.ActivationFunctionType.Square,
                            accum_out=t[:, bt:bt + 1])

                # ---- self-dots via PE gram diagonal (keeps DVE free) ----
                sdot = []
                for c, (i, j) in enumerate(COMBOS):
                    t = res_pool.tile([128, NBT], dt.float32, tag=f"sd{c}")
                    sdot.append(t)
                    for bt in range(NBT):
                        pg = pst_pool.tile([128, 128], dt.float32, tag="tp")
                        for k in range(2):
                            nc.tensor.matmul(
                                pg,
                                xT_r[i][:, k, bt * 128:(bt + 1) * 128],
                                xT_r[j][:, k, bt * 128:(bt + 1) * 128],
                                start=(k == 0), stop=(k == 1))
                        dg = scratch_pool.tile([128, 128], dt.float32, tag="diag")
                        nc.vector.tensor_mul(dg, pg, ident)
                        nc.vector.reduce_sum(t[:, bt:bt + 1], dg,
                                             axis=mybir.AxisListType.X)

                # ---- per-combo scale s = 1/(T*||x_i||*||x_j||) ----
                prod2 = res_pool.tile([128, NC_ * NBT], dt.float32, tag="prod2")
                for c, (i, j) in enumerate(COMBOS):
                    nc.vector.tensor_mul(prod2[:, c * NBT:(c + 1) * NBT],
                                         norm2[i], norm2[j])
                sqv = res_pool.tile([128, NC_ * NBT], dt.float32, tag="sqv")
                nc.scalar.activation(sqv, prod2,
                                     mybir.ActivationFunctionType.Sqrt,
                                     scale=T * T)
                nc.vector.reciprocal(sc_all, sqv)

            if stage >= 4:
                # ---- positive-logit column: exp(sdot * s) ----
                sexp = res_pool.tile([128, NBT, NC_], dt.float32, tag="sexp")
                for c in range(NC_):
                    for bt in range(NBT):
                        nc.scalar.activation(
                            sexp[:, bt, c:c + 1], sdot[c][:, bt:bt + 1],
                            mybir.ActivationFunctionType.Exp,
                            scale=sc_all[:, c * NBT + bt:c * NBT + bt + 1])
                nc.sync.dma_start(
                    selfexp.ap().rearrange("bt p c -> p bt c"), sexp)

            # ---- main loop over views and q-chunks of QH ----
            QH = 1024                  # q rows per iteration (2 PSUM banks)
            NQH = QC // QH
            for v in (range(NV) if stage >= 5 else ()):
                v_combos = [c for c, (i, j) in enumerate(COMBOS) if j == v]
                for qh in range(NQH):
                    qs = qh * QH
                    # natural-layout load: [128, g, d], row = p*8 + g so each
                    # partition is one contiguous 8 KB HBM packet. The q
                    # permutation propagates to the outs columns; the host
                    # gather undoes it.
                    mn = mnat_pool.tile([128, QH // 128, D], dt.float32, tag="mn")
                    nc.sync.dma_start(
                        mn, mem_in[v].ap()[qs:qs + QH, :]
                        .rearrange("(p g) d -> p g d", g=QH // 128))
                    # FIFO shift passthrough (host re-indexes by B rows)
                    nc.sync.dma_start(
                        newmem_chunk.ap()[v, qs:qs + QH, :]
                        .rearrange("(p g) d -> p g d", g=QH // 128), mn)
                    if stage < 6:
                        continue
                    # transpose to rhs [128 d_half, k, q] in f32r; 4 transposed
                    # blocks share one PSUM bank -> 1 cast per (k, 512-q-block)
                    rhs = rhs_pool.tile([128, 2, QH], dt.float32r, tag="rhs")
                    for qb2 in range(QH // 512):
                        for k in range(2):
                            pt = pst_pool.tile([128, 512], dt.float32, tag="tp")
                            for g in range(4):
                                nc.tensor.transpose(
                                    pt[:, g * 128:(g + 1) * 128],
                                    mn[:, qb2 * 4 + g, k * 128:(k + 1) * 128],
                                    ident)
                            nc.vector.tensor_copy(
                                rhs[:, k, qb2 * 512:(qb2 + 1) * 512], pt)
                    # matmuls + one batched exp + store per (combo, b-tile)
                    for c in (v_combos if stage >= 7 else ()):
                        i = COMBOS[c][0]
                        for bt in range(NBT):
                            pg = psmm_pool.tile([128, QH], dt.float32, tag="mm")
                            for qb2 in range(QH // 512):
                                for k in range(2):
                                    nc.tensor.matmul(
                                        pg[:, qb2 * 512:(qb2 + 1) * 512],
                                        xT_r[i][:, k, bt * 128:(bt + 1) * 128],
                                        rhs[:, k, qb2 * 512:(qb2 + 1) * 512],
                                        start=(k == 0), stop=(k == 1))
                            ob = osb_pool.tile([128, QH], dt.float32, tag="ob")
                            nc.scalar.activation(
                                ob, pg, mybir.ActivationFunctionType.Exp,
                                scale=sc_all[:, c * NBT + bt:c * NBT + bt + 1])
                            nc.sync.dma_start(
                                outs_chunk.ap()[c, bt * 128:(bt + 1) * 128,
                                                qs:qs + QH], ob)

    nc.compile()
    return nc


def _get_nc(stage=9):
    key = ("nc", stage)
    if key not in _CACHE:
        _CACHE[key] = _build(stage)
    return _CACHE[key]


def kernel(x0, x1, x2, x3, y0, y1, y2, y3, mem0, mem1, mem2, mem3,
           _run_kwargs=None, _return_raw=False, _stage=9):
    xs = [np.ascontiguousarray(np.asarray(a, dtype=np.float32))
          for a in (x0, x1, x2, x3)]
    ys = [np.ascontiguousarray(np.asarray(a, dtype=np.float32))
          for a in (y0, y1, y2, y3)]
    mems = [np.ascontiguousarray(np.asarray(a, dtype=np.float32)).reshape(Q, D)
            for a in (mem0, mem1, mem2, mem3)]

    nc = _get_nc(_stage)

    in_maps = []
    for k in range(N_CORES):
        m = {f"x{v}": xs[v] for v in range(NV)}
        for v in range(NV):
            m[f"mem{v}"] = np.ascontiguousarray(mems[v][k * QC:(k + 1) * QC])
        m["ytail"] = np.ascontiguousarray(
            np.stack([ys[v][k * YT:(k + 1) * YT] for v in range(NV)]))
        in_maps.append(m)

    res = bass_utils.run_bass_kernel_spmd(
        nc, in_maps, core_ids=list(range(N_CORES)), **(_run_kwargs or {}))

    # ---- gather ----
    outs = np.empty((NC_, B, 1 + Q), dtype=np.float32)
    outs[:, :, 0] = res.results[0]["selfexp"].reshape(B, NC_).T
    QH = 1024
    for k in range(N_CORES):
        chunk = res.results[k]["outs_chunk"]
        # undo per-1024-chunk column permutation: col g*128+p -> q p*8+g
        chunk = (chunk.reshape(NC_, B, QC // QH, 8, 128)
                 .swapaxes(3, 4).reshape(NC_, B, QC))
        outs[:, :, 1 + k * QC:1 + (k + 1) * QC] = chunk

    new_mem = np.empty((NV, 1, Q, D), dtype=np.float32)
    for k in range(N_CORES):
        chunk = res.results[k]["newmem_chunk"]  # mem rows [k*QC, (k+1)*QC)
        lo, hi = k * QC - B, (k + 1) * QC - B
        if lo < 0:
            new_mem[:, 0, 0:hi] = chunk[:, B:]
        else:
            new_mem[:, 0, lo:hi] = chunk
        yt = res.results[k]["ytail_out"]        # [NV, YT, D]
        new_mem[:, 0, Q - B + k * YT:Q - B + (k + 1) * YT] = yt

    if _return_raw:
        return (outs, new_mem), res
    return outs, new_mem


# revision 18
# speedup vs baseline: 1.0460x; 1.0460x over previous
"""MemoryMoCo (8-view combos + FIFO memory update) on 8 Trainium2 cores.

Sharding: over the queue dim Q. Core k holds rows [k*4096, (k+1)*4096) of
every memory bank. Per core:
  - all 8 combos (i,j): exp(scale * X_i @ Mem_j_slice^T) -> [8, 512, 4096]
  - positive-logit column exp(scale * <X_i, X_j>) (replicated; host takes core 0)
  - FIFO shift: streams its natural-layout mem tiles back out (rows shift by B
    on the host side of the gather), Y tail passes through the device.
Matmuls run in float32r (full-rate fp32 PE mode); norms/logit scaling in fp32.
"""
import numpy as np

import concourse.bass as bass
import concourse.tile as tile
import concourse.mybir as mybir
from concourse import bacc
from concourse import bass_utils
from concourse.masks import make_identity

dt = mybir.dt

N_CORES = 8
B, Q, D = 512, 32768, 256
T = 0.07
COMBOS = ((0, 1), (0, 2), (0, 3), (1, 0), (1, 2), (2, 0), (2, 1), (3, 0))
NC_ = len(COMBOS)
NV = 4                      # views
QC = Q // N_CORES           # 4096 queue rows per core
QB = 512                    # queue rows per main-loop iteration
NQB = QC // QB              # 8 iterations per view
NBT = B // 128              # 4 batch tiles
YT = B // N_CORES           # 64 Y-tail rows per core

_CACHE: dict = {}


def _build(stage=9):
    nc = bacc.Bacc("TRN2", target_bir_lowering=False, debug=False,
                   num_devices=N_CORES)

    x_in = [nc.dram_tensor(f"x{v}", [B, D], dt.float32, kind="ExternalInput")
            for v in range(NV)]
    mem_in = [nc.dram_tensor(f"mem{v}", [QC, D], dt.float32, kind="ExternalInput")
              for v in range(NV)]
    ytail_in = nc.dram_tensor("ytail", [NV, YT, D], dt.float32, kind="ExternalInput")

    outs_chunk = nc.dram_tensor("outs_chunk", [NC_, B, QC], dt.float32,
                                kind="ExternalOutput")
    selfexp = nc.dram_tensor("selfexp", [NBT, 128, NC_], dt.float32,
                             kind="ExternalOutput")
    newmem_chunk = nc.dram_tensor("newmem_chunk", [NV, QC, D], dt.float32,
                                  kind="ExternalOutput")
    ytail_out = nc.dram_tensor("ytail_out", [NV, YT, D], dt.float32,
                               kind="ExternalOutput")

    with tile.TileContext(nc) as tc:
        with (
            tc.tile_pool(name="const", bufs=1) as const_pool,
            tc.tile_pool(name="resident", bufs=1) as res_pool,
            tc.tile_pool(name="scratch", bufs=2) as scratch_pool,
            tc.tile_pool(name="mnat", bufs=6) as mnat_pool,
            tc.tile_pool(name="rhs", bufs=3) as rhs_pool,
            tc.tile_pool(name="osb", bufs=8) as osb_pool,
            tc.tile_pool(name="pst", bufs=2, space="PSUM") as pst_pool,
            tc.tile_pool(name="psmm", bufs=3, space="PSUM") as psmm_pool,
        ):
            ident = const_pool.tile([128, 128], dt.float32)
            make_identity(nc, ident)

            # ---- Y tail passthrough (tiny) ----
            yt_sb = res_pool.tile([YT, NV, D], dt.float32, tag="ytail")
            nc.sync.dma_start(yt_sb, ytail_in.ap().rearrange("v p d -> p v d"))
            nc.sync.dma_start(ytail_out.ap().rearrange("v p d -> p v d"), yt_sb)

            # ---- load X natural: [128, bt, d] with row = bt*128 + p ----
            x_sb = []
            for v in range(NV):
                t = res_pool.tile([128, NBT, D], dt.float32, tag=f"xnat{v}")
                nc.sync.dma_start(t, x_in[v].ap().rearrange("(bt p) d -> p bt d", p=128))
                x_sb.append(t)

            # ---- transpose X on PE -> xT_r [128 d_half, k, b] in f32r ----
            xT_r = []
            if stage >= 2:
                for v in range(NV):
                    t = res_pool.tile([128, 2, B], dt.float32r, tag=f"xtr{v}")
                    xT_r.append(t)
                    for k in range(2):
                        pt = pst_pool.tile([128, 512], dt.float32, tag="tp")
                        for bt in range(NBT):
                            nc.tensor.transpose(
                                pt[:, bt * 128:(bt + 1) * 128],
                                x_sb[v][:, bt, k * 128:(k + 1) * 128], ident)
                        nc.vector.tensor_copy(t[:, k, :], pt)

            # ---- norms^2 per view (ACT Square + row-accumulate) ----
            sc_all = res_pool.tile([128, NC_ * NBT], dt.float32, tag="scall")
            if stage >= 3:
                norm2 = []
                for v in range(NV):
                    t = res_pool.tile([128, NBT], dt.float32, tag=f"n2{v}")
                    norm2.append(t)
                    for bt in range(NBT):
                        sc = scratch_pool.tile([128, D], dt.float32, tag="ttr")
                        nc.scalar.activation(
                            sc, x_sb[v][:, bt, :],
                            mybir.ActivationFunctionType.Square,
                            accum_out=t[:, bt:bt + 1])

                # ---- self-dots: one DVE mul per combo + ACT row-accumulate ----
                sdot = []
                for c, (i, j) in enumerate(COMBOS):
                    t = res_pool.tile([128, NBT], dt.float32, tag=f"sd{c}")
                    sdot.append(t)
                    pr = scratch_pool.tile([128, NBT, D], dt.float32, tag="prod")
                    nc.vector.tensor_mul(pr, x_sb[i], x_sb[j])
                    for bt in range(NBT):
                        sc = scratch_pool.tile([128, D], dt.float32, tag="ttr")
                        nc.scalar.activation(
                            sc, pr[:, bt, :],
                            mybir.ActivationFunctionType.Copy,
                            accum_out=t[:, bt:bt + 1])

                # ---- per-combo scale s = 1/(T*||x_i||*||x_j||) ----
                prod2 = res_pool.tile([128, NC_ * NBT], dt.float32, tag="prod2")
                for c, (i, j) in enumerate(COMBOS):
                    nc.vector.tensor_mul(prod2[:, c * NBT:(c + 1) * NBT],
                                         norm2[i], norm2[j])
                sqv = res_pool.tile([128, NC_ * NBT], dt.float32, tag="sqv")
                nc.scalar.activation(sqv, prod2,
                                     mybir.ActivationFunctionType.Sqrt,
                                     scale=T * T)
                nc.vector.reciprocal(sc_all, sqv)

            if stage >= 4:
                # ---- positive-logit column: exp(sdot * s) ----
                sexp = res_pool.tile([128, NBT, NC_], dt.float32, tag="sexp")
                for c in range(NC_):
                    for bt in range(NBT):
                        nc.scalar.activation(
                            sexp[:, bt, c:c + 1], sdot[c][:, bt:bt + 1],
                            mybir.ActivationFunctionType.Exp,
                            scale=sc_all[:, c * NBT + bt:c * NBT + bt + 1])
                nc.sync.dma_start(
                    selfexp.ap().rearrange("bt p c -> p bt c"), sexp)

            # ---- main loop over views and q-chunks of QH ----
            QH = 1024                  # q rows per iteration (2 PSUM banks)
            NQH = QC // QH
            for v in (range(NV) if stage >= 5 else ()):
                v_combos = [c for c, (i, j) in enumerate(COMBOS) if j == v]
                for qh in range(NQH):
                    qs = qh * QH
                    # natural-layout load: [128, g, d], row = p*8 + g so each
                    # partition is one contiguous 8 KB HBM packet. The q
                    # permutation propagates to the outs columns; the host
                    # gather undoes it.
                    mn = mnat_pool.tile([128, QH // 128, D], dt.float32, tag="mn")
                    nc.sync.dma_start(
                        mn, mem_in[v].ap()[qs:qs + QH, :]
                        .rearrange("(p g) d -> p g d", g=QH // 128))
                    # FIFO shift passthrough (host re-indexes by B rows)
                    nc.sync.dma_start(
                        newmem_chunk.ap()[v, qs:qs + QH, :]
                        .rearrange("(p g) d -> p g d", g=QH // 128), mn)
                    if stage < 6:
                        continue
                    # transpose to rhs [128 d_half, k, q] in f32r; 4 transposed
                    # blocks share one PSUM bank -> 1 cast per (k, 512-q-block)
                    rhs = rhs_pool.tile([128, 2, QH], dt.float32r, tag="rhs")
                    for qb2 in range(QH // 512):
                        for k in range(2):
                            pt = pst_pool.tile([128, 512], dt.float32, tag="tp")
                            for g in range(4):
                                nc.tensor.transpose(
                                    pt[:, g * 128:(g + 1) * 128],
                                    mn[:, qb2 * 4 + g, k * 128:(k + 1) * 128],
                                    ident)
                            nc.vector.tensor_copy(
                                rhs[:, k, qb2 * 512:(qb2 + 1) * 512], pt)
                    # matmuls + one batched exp + store per (combo, b-tile)
                    for c in (v_combos if stage >= 7 else ()):
                        i = COMBOS[c][0]
                        for bt in range(NBT):
                            pg = psmm_pool.tile([128, QH], dt.float32, tag="mm")
                            for qb2 in range(QH // 512):
                                for k in range(2):
                                    nc.tensor.matmul(
                                        pg[:, qb2 * 512:(qb2 + 1) * 512],
                                        xT_r[i][:, k, bt * 128:(bt + 1) * 128],
                                        rhs[:, k, qb2 * 512:(qb2 + 1) * 512],
                                        start=(k == 0), stop=(k == 1))
                            ob = osb_pool.tile([128, QH], dt.float32, tag="ob")
                            nc.scalar.activation(
                                ob, pg, mybir.ActivationFunctionType.Exp,
                                scale=sc_all[:, c * NBT + bt:c * NBT + bt + 1])
                            nc.sync.dma_start(
                                outs_chunk.ap()[c, bt * 128:(bt + 1) * 128,
                                                qs:qs + QH], ob)

    nc.compile()
    return nc


def _get_nc(stage=9):
    key = ("nc", stage)
    if key not in _CACHE:
        _CACHE[key] = _build(stage)
    return _CACHE[key]


def kernel(x0, x1, x2, x3, y0, y1, y2, y3, mem0, mem1, mem2, mem3,
           _run_kwargs=None, _return_raw=False, _stage=9):
    xs = [np.ascontiguousarray(np.asarray(a, dtype=np.float32))
          for a in (x0, x1, x2, x3)]
    ys = [np.ascontiguousarray(np.asarray(a, dtype=np.float32))
          for a in (y0, y1, y2, y3)]
    mems = [np.ascontiguousarray(np.asarray(a, dtype=np.float32)).reshape(Q, D)
            for a in (mem0, mem1, mem2, mem3)]

    nc = _get_nc(_stage)

    in_maps = []
    for k in range(N_CORES):
        m = {f"x{v}": xs[v] for v in range(NV)}
        for v in range(NV):
            m[f"mem{v}"] = np.ascontiguousarray(mems[v][k * QC:(k + 1) * QC])
        m["ytail"] = np.ascontiguousarray(
            np.stack([ys[v][k * YT:(k + 1) * YT] for v in range(NV)]))
        in_maps.append(m)

    res = bass_utils.run_bass_kernel_spmd(
        nc, in_maps, core_ids=list(range(N_CORES)), **(_run_kwargs or {}))

    # ---- gather ----
    outs = np.empty((NC_, B, 1 + Q), dtype=np.float32)
    outs[:, :, 0] = res.results[0]["selfexp"].reshape(B, NC_).T
    QH = 1024
    for k in range(N_CORES):
        chunk = res.results[k]["outs_chunk"]
        # undo per-1024-chunk column permutation: col g*128+p -> q p*8+g
        chunk = (chunk.reshape(NC_, B, QC // QH, 8, 128)
                 .swapaxes(3, 4).reshape(NC_, B, QC))
        outs[:, :, 1 + k * QC:1 + (k + 1) * QC] = chunk

    new_mem = np.empty((NV, 1, Q, D), dtype=np.float32)
    for k in range(N_CORES):
        chunk = res.results[k]["newmem_chunk"]  # mem rows [k*QC, (k+1)*QC)
        lo, hi = k * QC - B, (k + 1) * QC - B
        if lo < 0:
            new_mem[:, 0, 0:hi] = chunk[:, B:]
        else:
            new_mem[:, 0, lo:hi] = chunk
        yt = res.results[k]["ytail_out"]        # [NV, YT, D]
        new_mem[:, 0, Q - B + k * YT:Q - B + (k + 1) * YT] = yt

    if _return_raw:
        return (outs, new_mem), res
    return outs, new_mem
